# revision 3
# baseline (speedup 1.0000x reference)
"""Sparse (mean-thresholded) attention TRN2 kernel, v4.

Fully-pipelined single-phase design. ACT (exp) is the wall at ~33.3us;
everything else is tucked underneath it:

  - S slabs [128 i, 2048 j] in a bufs=2 PSUM rotation, exp+accum on ACT.
  - mask (e0>t)*r fused into tensor_scalar op1 (4x) + tensor_tensor mult
    (2x) on DVE; the masked matrix carries the 1/s_i scale.
  - DMA transpose feeds PV as lhsT: out[i-part, d-free] -> 16 matmuls x
    64-free = 427ns/tile on PE (half the old orientation's cost).
  - PV rides PSUM "steal-pairs": two consecutive pool allocs preserve
    the A/B parity so the S cadence (and thus ACT) never bubbles. Only
    the PV matmul + one Pool evacuation sit on the slab-reuse chain.
  - outputs are PE-transposed ([128,64] -> [64,128]) into oT [64, N]
    so the final DMA uses 64 fat descriptors; all transposes batch into
    the post-S15 steals + drain where nothing gates the S cadence.

Fill: K/Q/V^T projections chunk-pipelined behind the 4 xt chunk DMAs
(all operands f32r via bitcast DMAs, 1 cycle/row), psum->SBUF copies
spread over ACT/DVE/Pool, V_bf via one 3D DMA transpose, exp table
pre-warmed with a dummy activation at t~0.
"""

import sys

sys.path.insert(0, "/opt/trn_rl_repo")

import numpy as np

import concourse.bacc as bacc
import concourse.tile as tile
from concourse import mybir

f32 = mybir.dt.float32
f32r = mybir.dt.float32r
bf16 = mybir.dt.bfloat16
AF = mybir.ActivationFunctionType
OP = mybir.AluOpType

B, N, D = 8, 2048, 64
P = 128
NT = N // P          # 16 i-slabs / j-tiles
C_SHIFT = 60.0

_NC = None


def _make_identity(nc, identity):
    nc.gpsimd.memset(identity, 0.0)
    nc.gpsimd.affine_select(
        out=identity,
        in_=identity,
        compare_op=OP.not_equal,
        fill=1.0,
        base=0,
        pattern=[[-1, identity.shape[0]]],
        channel_multiplier=1,
    )


def _build():
    nc = bacc.Bacc(None, target_bir_lowering=False)

    # x^T augmented with a ones row (built host-side): [D+1, N]
    xt_d = nc.dram_tensor("xt", [D + 1, N], f32, kind="ExternalInput")
    # packed weights: rows 0-63 = W, row 64 = bias; cols [Wq | Wk | Wv]
    w_d = nc.dram_tensor("w", [D + 1, 3 * D], f32, kind="ExternalInput")
    o_raw_d = nc.dram_tensor("oraw", [P, NT * D], bf16, kind="ExternalOutput")
    s_dbg = nc.dram_tensor("sdbg", [P, NT], f32, kind="ExternalOutput")
    kt_dbg = nc.dram_tensor("ktdbg", [D, 512], f32, kind="ExternalOutput")
    vb_dbg = nc.dram_tensor("vbdbg", [P, 8 * D], bf16, kind="ExternalOutput")
    e0_dbg = nc.dram_tensor("e0dbg", [P, N], bf16, kind="ExternalOutput")

    with tile.TileContext(nc) as tc:
        with (
            tc.tile_pool(name="sing", bufs=1) as sing,
            tc.tile_pool(name="e0p", bufs=6) as e0p,
            tc.tile_pool(name="ps", bufs=2, space="PSUM") as ps,
        ):
            # ---------------- static SBUF ----------------
            junk = sing.tile([P, 16], bf16)
            nc.gpsimd.memset(junk, 0.25)
            nc.gpsimd.dma_start(s_dbg[:, 0:8], junk[:, 0:16].bitcast(f32))
            ebias = sing.tile([P, 1], f32)
            nc.vector.memset(ebias, -C_SHIFT)
            # dummy exp: pulls the ACT table load off the critical path
            dummy_e = sing.tile([P, 2], f32)
            nc.scalar.activation(
                out=dummy_e[:, 0:1], in_=ebias[:, 0:1], func=AF.Exp,
                bias=0.0, scale=0.0,
            )

            # inputs straight to f32r (same bits; matmul wants f32r rate)
            w_r = sing.tile([D + 1, 3 * D], f32r)
            nc.sync.dma_start(w_r, w_d[:].bitcast(f32r))
            xTf = sing.tile([D + 1, N], f32r)
            for q in range(4):
                nc.sync.dma_start(
                    xTf[:, q * 512 : (q + 1) * 512],
                    xt_d[:, q * 512 : (q + 1) * 512].bitcast(f32r),
                )

            # per-chunk K/Q tiles decouple the copy->S dependencies
            KTc = [sing.tile([D, 512], f32r, name=f"kt{c}") for c in range(4)]
            QTc = [sing.tile([D, 512], f32r, name=f"qt{c}") for c in range(4)]
            VTb = [sing.tile([D, N // 2], bf16, name=f"vtb{h}") for h in range(2)]
            s_all = sing.tile([P, NT], f32)
            t_all = sing.tile([P, NT], f32)
            r_all = sing.tile([P, NT], f32)
            V_bf = [sing.tile([P, NT // 2, D], bf16, name=f"vbf{h}") for h in range(2)]
            MT = [
                sing.tile([P, NT, P], bf16, name=f"mt{it}") for it in range(NT - 1)
            ]
            # slab 15's transpose lands chunked: 4 independent quarter tiles
            MT15q = [sing.tile([P, 4, P], bf16, name=f"mt15q{c}") for c in range(4)]
            # slab 15's masked chunks: separate tiles so the chunked DMA
            # transposes never WAR-chain against later chunk masks
            m15c = [sing.tile([P, 512], bf16, name=f"m15c{c}") for c in range(4)]
            o_bf = sing.tile([P, NT * D], bf16)   # PV outs [i-part, d]

            def vbf_jt(jt):
                return V_bf[jt // 8][:, jt % 8, :]

            def slot():
                return ps.tile([P, N], f32, tag="S", name="pslab")

            # ---------------- fill: projections ----------------
            # two paired allocs: [K01|Q01] then [K23|Q23]; K at cols 0:1024,
            # Q at cols 1024:2048 (partition rows 0:64)
            def proj_pair(pa, clo):
                for c in (clo, clo + 1):
                    nc.tensor.matmul(
                        pa[0:D, (c - clo) * 512 : (c - clo + 1) * 512],
                        w_r[:, D : 2 * D],
                        xTf[:, c * 512 : (c + 1) * 512],
                        start=True, stop=True,
                    )
                    nc.tensor.matmul(
                        pa[0:D, 1024 + (c - clo) * 512 : 1024 + (c - clo + 1) * 512],
                        w_r[:, 0:D],
                        xTf[:, c * 512 : (c + 1) * 512],
                        start=True, stop=True,
                    )

            p1 = slot()
            for j in range(2):  # PE clock warmup
                nc.tensor.matmul(
                    p1[64:65, 1920 + 16 * j : 1936 + 16 * j],
                    junk[:, j : j + 1], junk[:, 0:16],
                    start=True, stop=True,
                )
            proj_pair(p1, 0)
            p2 = slot()
            proj_pair(p2, 2)
            # copies spread over ACT/DVE/Pool; p2's gate S0 (slot reuse)
            nc.scalar.copy(KTc[0][:, :], p1[0:D, 0:512])
            nc.vector.tensor_copy(KTc[1][:, :], p1[0:D, 512:1024])
            nc.vector.tensor_copy(QTc[0][:, :], p1[0:D, 1024:1536])
            nc.scalar.copy(QTc[1][:, :], p1[0:D, 1536:2048])
            nc.scalar.copy(KTc[2][:, :], p2[0:D, 0:512])
            nc.vector.tensor_copy(KTc[3][:, :], p2[0:D, 512:1024])
            nc.scalar.copy(QTc[2][:, :], p2[0:D, 1024:1536])
            nc.vector.tensor_copy(QTc[3][:, :], p2[0:D, 1536:2048])
            # V^T projection as a third fill alloc (Pool cannot read PSUM,
            # and DVE has no slack once the mask stream starts)
            vp = slot()
            for c in range(4):
                nc.tensor.matmul(
                    vp[0:D, c * 512 : (c + 1) * 512],
                    w_r[:, 2 * D : 3 * D],
                    xTf[:, c * 512 : (c + 1) * 512],
                    start=True, stop=True,
                )
            nc.vector.tensor_copy(VTb[0][:, 0:1024], vp[0:D, 0:1024])
            nc.vector.tensor_copy(VTb[1][:, 0:1024], vp[0:D, 1024:2048])
            nc.sync.dma_start_transpose(V_bf[0][:, :, :], VTb[0][:, 0 : N // 2])
            nc.sync.dma_start_transpose(V_bf[1][:, :, :], VTb[1][:, 0 : N // 2])

            # ---------------- per-slab helpers ----------------
            sp_tiles = [None] * NT

            def s_matmul(it):
                sp = slot()
                sp_tiles[it] = sp
                qsl = QTc[it // 4][:, (it % 4) * P : (it % 4 + 1) * P]
                for c in range(4):
                    nc.tensor.matmul(
                        sp[:, c * 512 : (c + 1) * 512],
                        qsl,
                        KTc[c][:, :],
                        start=True, stop=True,
                    )
                return sp

            def exp_slab(sp, it, e0):
                nc.scalar.activation(
                    out=e0[:, 0:N],
                    in_=sp[:, 0:N],
                    func=AF.Exp,
                    bias=ebias,
                    scale=1.0,
                    accum_out=s_all[:, it : it + 1],
                )

            def thresh(it):
                nc.vector.tensor_scalar(
                    out=t_all[:, it : it + 1],
                    in0=s_all[:, it : it + 1],
                    scalar1=1.0 / N,
                    scalar2=None,
                    op0=OP.mult,
                )
                nc.vector.reciprocal(r_all[:, it : it + 1], s_all[:, it : it + 1])

            def mask_chunk(it, e0, lo, hi, out=None):
                # msk = (e0 > t) * r ; out = e0 * msk  (carries the 1/s scale)
                msk = e0p.tile([P, N], bf16, tag="msk", bufs=3, name="msk")
                if out is None:
                    mfull = e0p.tile([P, N], bf16, tag="mfull", bufs=3, name="mf")
                    dst = mfull[:, lo:hi]
                else:
                    mfull = None
                    dst = out
                nc.vector.tensor_scalar(
                    out=msk[:, lo:hi],
                    in0=e0[:, lo:hi],
                    scalar1=t_all[:, it : it + 1],
                    scalar2=r_all[:, it : it + 1],
                    op0=OP.is_gt,
                    op1=OP.mult,
                )
                nc.vector.tensor_tensor(
                    out=dst, in0=e0[:, lo:hi], in1=msk[:, lo:hi],
                    op=OP.mult,
                )
                return mfull

            def pv_new(pslab, col, t):
                # out[i-part, d] for i-tile t into pslab f32 cols [col:col+64]
                for jt in range(NT):
                    nc.tensor.matmul(
                        pslab[:, col : col + 64],
                        MT[t][:, jt, :],
                        vbf_jt(jt),
                        start=(jt == 0),
                        stop=(jt == NT - 1),
                    )

            def obf_copy(pslab, col, t):
                nc.vector.tensor_copy(
                    o_bf[:, t * D : (t + 1) * D], pslab[:, col : col + 64]
                )

            # ---------------- main pipeline ----------------
            e0_tiles = [None] * NT

            def emit_slab_head(it):
                sp = s_matmul(it)
                e0 = e0p.tile([P, N], bf16, tag="e0", name="e0")
                e0_tiles[it] = e0
                exp_slab(sp, it, e0)

            emit_slab_head(0)
            emit_slab_head(1)
            thresh(0)
            mf0 = mask_chunk(0, e0_tiles[0], 0, N)
            nc.sync.dma_start_transpose(MT[0][:, :, :], mf0[:, 0:N])

            for it in range(2, NT):
                emit_slab_head(it)
                jt_prev = it - 1
                thresh(jt_prev)
                mfp = mask_chunk(jt_prev, e0_tiles[jt_prev], 0, N)
                nc.sync.dma_start_transpose(
                    MT[jt_prev][:, :, :], mfp[:, 0:N]
                )
                # PV(t) piggybacks into the slab exp(it-1) just vacated
                t = it - 5
                if 0 <= t <= 10:
                    pv_new(sp_tiles[it - 1], 0, t)
                    obf_copy(sp_tiles[it - 1], 0, t)

            # ---------------- drain ----------------
            e015 = e0_tiles[15]

            # slab-15 mask: r-folded 2-op chunks on DVE writing into per-chunk
            # tiles; each chunk's transpose goes straight to its MT15 quarter
            thresh(NT - 1)
            for c in range(4):
                mask_chunk(NT - 1, e015, c * 512, (c + 1) * 512,
                           out=m15c[c][:, :])
                nc.sync.dma_start_transpose(MT15q[c][:, :, :], m15c[c][:, :])

            # d2 (slab14's slot, free during exp15): PV 10..13
            d2 = slot()
            pv_new(d2, 0, 10)
            pv_new(d2, 64, 11)
            pv_new(d2, 128, 12)
            pv_new(d2, 192, 13)
            obf_copy(d2, 0, 10)
            obf_copy(d2, 64, 11)
            obf_copy(d2, 128, 12)
            obf_copy(d2, 192, 13)

            # d3 (slab15's slot): PV14 + chunk-chasing PV15
            d3 = slot()
            pv_new(d3, 0, 14)
            for g in range(4):
                for jt in range(4 * g, 4 * g + 4):
                    nc.tensor.matmul(
                        d3[:, 64:128],
                        MT15q[jt // 4][:, jt % 4, :],
                        vbf_jt(jt),
                        start=(jt == 0),
                        stop=(jt == NT - 1),
                    )
            obf_copy(d3, 0, 14)
            obf_copy(d3, 64, 15)
            nc.sync.dma_start(o_raw_d[:, :], o_bf[:, :])
            nc.sync.dma_start(kt_dbg[:, 0:16], KTc[0][:, 0:16].bitcast(f32))
            nc.sync.dma_start(vb_dbg[:, 0:16], V_bf[0][:, 0, 0:16])
            nc.sync.dma_start(e0_dbg[:, 0:16], m15c[0][:, 0:16])

    nc.compile()
    return nc


def _get_nc():
    global _NC
    if _NC is None:
        _NC = _build()
    return _NC


_RUNNER = None


def _get_runner():
    """Build (once) a cached jitted SPMD executor for the bass module."""
    global _RUNNER
    if _RUNNER is not None:
        return _RUNNER

    import jax
    from jax.sharding import Mesh, PartitionSpec
    from jax.experimental.shard_map import shard_map
    from concourse import mybir as _mb
    from concourse.bass2jax import (
        _bass_exec_p,
        install_neuronx_cc_hook,
        partition_id_tensor,
    )

    nc = _get_nc()
    install_neuronx_cc_hook()

    partition_name = nc.partition_id_tensor.name if nc.partition_id_tensor else None
    in_names, out_names, out_avals, out_shapes = [], [], [], []
    for alloc in nc.m.functions[0].allocations:
        if not isinstance(alloc, _mb.MemoryLocationSet):
            continue
        name = alloc.memorylocations[0].name
        if alloc.kind == "ExternalInput":
            if name != partition_name:
                in_names.append(name)
        elif alloc.kind == "ExternalOutput":
            out_names.append(name)
            shape = tuple(alloc.tensor_shape)
            dtype = _mb.dt.np(alloc.dtype)
            out_avals.append(jax.core.ShapedArray(shape, dtype))
            out_shapes.append((shape, dtype))
    n_params = len(in_names)
    n_outs = len(out_avals)
    all_in_names = list(in_names) + list(out_names)
    if partition_name is not None:
        all_in_names.append(partition_name)

    def _body(*args):
        operands = list(args)
        if partition_name is not None:
            operands.append(partition_id_tensor())
        outs = _bass_exec_p.bind(
            *operands,
            out_avals=tuple(out_avals),
            in_names=tuple(all_in_names),
            out_names=tuple(out_names),
            lowering_input_output_aliases=(),
            sim_require_finite=True,
            sim_require_nnan=True,
            nc=nc,
        )
        return tuple(outs)

    devices = jax.devices()[:B]
    mesh = Mesh(np.asarray(devices), ("core",))
    in_specs = (PartitionSpec("core"),) * (n_params + n_outs)
    out_specs = (PartitionSpec("core"),) * n_outs
    donate = tuple(range(n_params, n_params + n_outs))
    sharded = jax.jit(
        shard_map(
            _body, mesh=mesh, in_specs=in_specs, out_specs=out_specs, check_rep=False
        ),
        donate_argnums=donate,
        keep_unused=True,
    )

    def run(in_maps):
        concat_in = [
            np.concatenate([np.asarray(m[name]) for m in in_maps], axis=0)
            for name in in_names
        ]
        zero_outs = [
            np.zeros((B * shape[0], *shape[1:]), dtype) for shape, dtype in out_shapes
        ]
        outs = sharded(*concat_in, *zero_outs)
        outs = [np.asarray(o) for o in outs]
        results = []
        for c in range(B):
            r = {}
            for i, name in enumerate(out_names):
                d0 = out_shapes[i][0][0]
                r[name] = outs[i][c * d0 : (c + 1) * d0]
            results.append(r)
        return results

    _RUNNER = run
    return _RUNNER


def kernel(x, Wq, bq, Wk, bk, Wv, bv):
    x = np.ascontiguousarray(np.asarray(x, dtype=np.float32))
    w_all = np.zeros((D + 1, 3 * D), dtype=np.float32)
    w_all[:D, 0:D] = np.asarray(Wq, np.float32)
    w_all[D, 0:D] = np.asarray(bq, np.float32)
    w_all[:D, D : 2 * D] = np.asarray(Wk, np.float32)
    w_all[D, D : 2 * D] = np.asarray(bk, np.float32)
    w_all[:D, 2 * D : 3 * D] = np.asarray(Wv, np.float32)
    w_all[D, 2 * D : 3 * D] = np.asarray(bv, np.float32)

    ones_row_np = np.ones((1, N), dtype=np.float32)
    xts = [
        np.ascontiguousarray(
            np.concatenate([x[b].T.astype(np.float32), ones_row_np], axis=0)
        )
        for b in range(B)
    ]
    run = _get_runner()
    in_maps = [{"xt": xts[b], "w": w_all} for b in range(B)]
    results = run(in_maps)

    out = np.empty((B, N, D), dtype=np.float32)
    for b in range(B):
        oraw = results[b]["oraw"].astype(np.float32)      # [128, 16*64]
        out[b] = (
            oraw.reshape(P, NT, D).transpose(1, 0, 2).reshape(N, D)
        )
    return out
